# revision 18
# baseline (speedup 1.0000x reference)
"""Distributed Trainium2 kernel for nn_Attention (B=2, N=2048, D=1024, H=16).

Sharding: tensor-parallel over heads (2 heads per core) for qkv + attention,
then an AllToAll redistributes attention output so each core projects a
512-row slice of the output (cores 0-3: batch 0, cores 4-7: batch 1).

Structure (v6): the exp stream on ScalarE is the attention phase's hard
floor (~1.01us per [128,1024] block, 128 blocks); the Tile scheduler lays
the program out as [init | qkv(b0)+most of qkv(b1) dense | ACT-bound
attention | AllToAll | proj]. Measured lessons folded in:
  - Forcing qkv matmuls between the PV pairs slows the ACT stream ~20%
    (contention), so qkv work is emitted as coarse trailing items and the
    scheduler hoists it into the dense phase.
  - Both heads' softmax denominators land on psum partitions 0/32 (head-B
    ones column at 160), so one DVE reciprocal [33,512] covers both heads.
  - Iteration-boundary ACT bubbles come from the single-buffered PV
    accumulators: copies are ordered denA,rawA,denB,rawB so oA frees after
    two DVE ops; the final iteration's raw copies go to the idle ScalarE.
  - The ncfw warm-up AllGather is pinned late (input DMA sources
    iteration-6 data) so the real AllToAll starts hot.
  - A paced dummy-matmul chain bridges the AllToAll so proj doesn't run
    at the 1.2GHz throttled clock.
"""

import sys
import types

import numpy as np

if "/opt/trn_rl_repo" not in sys.path:
    sys.path.insert(0, "/opt/trn_rl_repo")

import ml_dtypes

B, N, D = 2, 2048, 1024
H, HD = 16, 64
SCALE = HD**-0.5
TOK = B * N  # 4096, token index = b*N + t
EC = 8  # embed-dim chunks of 128
NCORES = 8
# per k-block vones layout [128 tok, 256]:
#   [1 | 0*63 | V_A(64) | 0*32 | 1 | 0*31 | V_B(64)]
# so the PV matmul (M=128) puts head-A's softmax denominator on psum
# partition 0 and head-B's on partition 32.
VSTRIDE = 256
NKB = TOK // 128  # 32 k-blocks across both batches
NTCN = TOK // 512  # 8 qkv token tiles

BF16 = ml_dtypes.bfloat16


def _install_axon_profile_hook():
    """Best-effort: register the NTFF profile hook the RL container's antenv
    stub omits, so run_bass_kernel_spmd(trace=True) can report exec_time_ns."""
    try:
        import antenv

        if "antenv.axon_hooks" not in sys.modules:
            hooks = types.ModuleType("antenv.axon_hooks")
            hooks._hook = None
            hooks.set_axon_ntff_profile_hook = lambda h: setattr(hooks, "_hook", h)
            hooks.get_axon_ntff_profile_hook = lambda: hooks._hook
            sys.modules["antenv.axon_hooks"] = hooks
            antenv.axon_hooks = hooks
            from trn_agent_boot.trn_boot import _ntff_profile_via_ctypes

            hooks.set_axon_ntff_profile_hook(
                _ntff_profile_via_ctypes("/opt/axon/libaxon_pjrt.so")
            )
        return True
    except Exception:
        return False


def _split_multi_waits(nc):
    """neuronxcc's walrus (CoreV3 setupSyncWait) rejects instructions that
    carry more than one semaphore wait, but Tile's wait assignment freely
    attaches several. Hoist the extra waits onto freshly inserted same-engine
    NoOps placed directly before the instruction — the engine stalls at the
    same program point, so semantics are unchanged."""
    import concourse.mybir as mybir

    n_split = 0
    for fn in nc.m.functions:
        for bb in fn.blocks:
            insts = bb.instructions
            if not any(
                i.sync_info is not None and len(i.sync_info.on_wait) > 1
                for i in insts
            ):
                continue
            new_insts = []
            for ins in insts:
                si = ins.sync_info
                if si is not None and len(si.on_wait) > 1:
                    waits = list(si.on_wait)
                    for w in waits[:-1]:
                        nop = mybir.InstNoOp(
                            name=f"wsplit-{n_split}",
                            engine=ins.engine,
                            ins=[],
                            outs=[],
                            sync_info=mybir.SyncInfo(on_wait=[w], on_update=[]),
                        )
                        new_insts.append(nop)
                        n_split += 1
                    ins.sync_info = mybir.SyncInfo(
                        on_wait=[waits[-1]], on_update=list(si.on_update)
                    )
                new_insts.append(ins)
            bb.instructions = new_insts


def _build_nc():
    import concourse.bass as bass
    import concourse.mybir as mybir
    import concourse.tile as tile

    F32 = mybir.dt.float32
    BF = mybir.dt.bfloat16
    AF = mybir.ActivationFunctionType
    ALU = mybir.AluOpType

    nc = bass.Bass()
    xT_ext = nc.declare_dram_parameter("xT", [D, TOK], BF, isOutput=False)
    wq_ext = nc.declare_dram_parameter("wq", [128, 1024], BF, isOutput=False)
    wk_ext = nc.declare_dram_parameter("wk", [128, 1024], BF, isOutput=False)
    wv_ext = nc.declare_dram_parameter("wv", [128, 1024], BF, isOutput=False)
    wp_ext = nc.declare_dram_parameter("wp", [128, 8192], BF, isOutput=False)
    bias_ext = nc.declare_dram_parameter("bias", [128, 8], F32, isOutput=False)
    coreid_ext = nc.declare_dram_parameter(
        "coreid", [1, 1], mybir.dt.uint32, isOutput=False
    )
    out_ext = nc.declare_dram_parameter("out", [D, 512], F32, isOutput=True)

    with tile.TileContext(nc) as tc:
        with (
            tc.tile_pool(name="const", bufs=1) as cpool,
            tc.tile_pool(name="e", bufs=6) as epool,
            tc.tile_pool(name="norm", bufs=2) as npool,
            tc.tile_pool(name="y", bufs=2) as ypool,
            tc.tile_pool(name="psum", bufs=2, space="PSUM") as psum,
            tc.tile_pool(name="dram", bufs=1, space="DRAM") as dram,
        ):
            wq_sb = cpool.tile([128, 1024], BF)
            wk_sb = cpool.tile([128, 1024], BF)
            wv_sb = cpool.tile([128, 1024], BF)
            wp_sb = cpool.tile([128, 8192], BF)
            bias_sb = cpool.tile([128, 8], F32)
            qt_sb = cpool.tile([128, TOK], BF)
            kt_sb = cpool.tile([128, TOK], BF)
            vones = cpool.tile([128, NKB, VSTRIDE], BF)
            x_sb = [
                cpool.tile([128, EC, 512], BF, name=f"x{t}") for t in range(NTCN)
            ]
            garb = cpool.tile([128, 512], BF)
            gout = cpool.tile([128, 512], BF)

            nc.vector.memset(garb[:], 0.0)
            nc.vector.memset(vones[:], 0.0)
            nc.vector.memset(vones[:, :, 0:1], 1.0)
            nc.vector.memset(vones[:, :, 160:161], 1.0)
            bones = cpool.tile([33, 64], BF)
            nc.vector.memset(bones[0:1, :], 1.0)
            nc.vector.memset(bones[32:33, :], 1.0)

            # ---------------- input DMA issue schedule ----------------
            # sync gets the weights + even x(b0) chunks; scalar (idle until
            # the first exp) gets the odd chunks. x(b1) is issued from
            # inside attention iterations 0-1 on sync; wp/bias mid-attention
            # (the prologue window is HBM-bandwidth limited: 8 cores pull
            # replicated x concurrently).
            nc.sync.dma_start(wq_sb[:, 0:512], wq_ext[:, 0:512])
            nc.sync.dma_start(wq_sb[:, 512:1024], wq_ext[:, 512:1024])
            for ec in range(0, EC, 2):
                nc.sync.dma_start(
                    x_sb[0][:, ec, :], xT_ext[ec * 128 : (ec + 1) * 128, 0:512]
                )
            nc.sync.dma_start(wk_sb[:], wk_ext[:])
            nc.sync.dma_start(wv_sb[:], wv_ext[:])
            for tcn in range(1, 4):
                for ec in range(0, EC, 2):
                    nc.sync.dma_start(
                        x_sb[tcn][:, ec, :],
                        xT_ext[ec * 128 : (ec + 1) * 128, tcn * 512 : (tcn + 1) * 512],
                    )
            for tcn in range(4):
                for ec in range(1, EC, 2):
                    nc.scalar.dma_start(
                        x_sb[tcn][:, ec, :],
                        xT_ext[ec * 128 : (ec + 1) * 128, tcn * 512 : (tcn + 1) * 512],
                    )

            # ---------------- qkv emission helpers ----------------
            def emit_QK(t, wsb, dst):
                ps = psum.tile([128, 1024], F32, tag="spair", bufs=3)
                for ec in range(EC):
                    nc.tensor.matmul(
                        ps[:, 0:512],
                        wsb[:, ec * 128 : (ec + 1) * 128],
                        x_sb[t][:, ec, :],
                        start=(ec == 0),
                        stop=(ec == EC - 1),
                    )
                nc.vector.tensor_copy(dst[:, t * 512 : (t + 1) * 512], ps[:, 0:512])

            def emit_Q(t):
                emit_QK(t, wq_sb, qt_sb)

            def emit_K(t):
                emit_QK(t, wk_sb, kt_sb)

            def emit_V(t):
                ps = psum.tile([128, 1024], F32, tag="spair", bufs=3)
                for tsub in range(4):
                    for ec in range(EC):
                        nc.tensor.matmul(
                            ps[:, tsub * 128 : tsub * 128 + 128],
                            x_sb[t][:, ec, tsub * 128 : (tsub + 1) * 128],
                            wv_sb[:, ec * 128 : (ec + 1) * 128],
                            start=(ec == 0),
                            stop=(ec == EC - 1),
                        )
                for tsub in range(4):
                    g = t * 4 + tsub
                    nc.vector.tensor_copy(
                        vones[:, g, 64:128], ps[:, tsub * 128 : tsub * 128 + 64]
                    )
                    nc.vector.tensor_copy(
                        vones[:, g, 192:256], ps[:, tsub * 128 + 64 : tsub * 128 + 128]
                    )

            # ---------------- prologue: minimal batch-0 qkv ----------------
            emit_Q(0)
            emit_K(0)
            emit_V(0)
            emit_K(1)
            emit_V(1)

            # feeder schedule: (iter, kb) -> list of qkv emitters, placed as
            # coarse trailing items; the Tile scheduler hoists them into the
            # dense region where the PE has slack.
            FEED = {
                (0, 0): [lambda: emit_K(2)],
                (0, 3): [lambda: emit_V(2)],
                (0, 6): [lambda: emit_K(3)],
                (0, 9): [lambda: emit_V(3)],
                (0, 13): [lambda: emit_Q(1)],
                (1, 0): [lambda: emit_Q(2)],
                (1, 4): [lambda: emit_K(4)],
                (1, 8): [lambda: emit_V(4)],
                (1, 12): [lambda: emit_Q(3)],
                (2, 0): [lambda: emit_K(5)],
                (2, 4): [lambda: emit_V(5)],
                (2, 8): [lambda: emit_K(6)],
                (3, 0): [lambda: emit_K(7)],
                (3, 4): [lambda: emit_V(6)],
                (3, 8): [lambda: emit_Q(4)],
                (4, 0): [lambda: emit_V(7)],
                (4, 8): [lambda: emit_Q(5)],
                (5, 6): [lambda: emit_Q(6)],
                (6, 6): [lambda: emit_Q(7)],
            }
            # x(b1) chunk DMAs issued on sync from iters 0-1
            XB1 = {}
            for i, t in enumerate(range(4, 8)):
                for ec in range(EC):
                    slot = i * EC + ec  # 0..31 over iters 0-1
                    XB1.setdefault((slot // 16, slot % 16), []).append((t, ec))

            # ---------------- attention ----------------
            a2a_in = dram.tile([1024, 512], BF)
            a2a_out = dram.tile([1024, 512], BF)
            warm_in = dram.tile([1, 512], BF)
            warm_out = dram.tile([8, 512], BF)

            def emit_norm_head(pend, j, rec_in):
                """Normalize one head of a finished (b, qb) iteration's raw
                attention output; overlapped with the next iteration."""
                pb, pqb, raws, _den = pend
                p0 = 32 * j
                bcp = psum.tile([128, 512], F32, tag="spair", bufs=3)
                nc.tensor.matmul(
                    bcp[64:128, :],
                    bones[p0 : p0 + 1, 0:64],
                    rec_in[p0 : p0 + 1, :],
                    start=True,
                    stop=True,
                )
                onorm = npool.tile([128, 512], BF, tag="onorm")
                nc.vector.tensor_mul(
                    onorm[64:128, :], raws[j][64:128, :], bcp[64:128, :]
                )
                row = 128 * (4 * pb + pqb) + 64 * j
                nc.sync.dma_start(a2a_in[row : row + 64, :], onorm[64:128, :])

            def emit_scores(b, qb, kb):
                qoff = b * N + qb * 512
                koff = b * N + kb * 128
                sp = psum.tile([128, 1024], F32, tag="spair", bufs=3)
                nc.tensor.matmul(
                    sp[:, 0:512],
                    kt_sb[0:64, koff : koff + 128],
                    qt_sb[0:64, qoff : qoff + 512],
                    start=True,
                    stop=True,
                )
                nc.tensor.matmul(
                    sp[:, 512:1024],
                    kt_sb[64:128, koff : koff + 128],
                    qt_sb[64:128, qoff : qoff + 512],
                    start=True,
                    stop=True,
                )
                e_t = epool.tile([128, 1024], BF)
                nc.scalar.activation(e_t[:], sp[:], AF.Exp, scale=SCALE)
                return e_t

            iters = [(b, qb) for b in range(B) for qb in range(N // 512)]
            pending = None
            e_carry = None
            rec_cur = None
            warm_src = None
            for it_idx, (b, qb) in enumerate(iters):
                oA = psum.tile([128, 512], F32, tag="oA", bufs=1)
                oB = psum.tile([128, 512], F32, tag="oB", bufs=1)
                for kb in range(N // 128):
                    g = b * (N // 128) + kb
                    if kb == 0:
                        if e_carry is not None:
                            e_t = e_carry
                            e_carry = None
                        else:
                            e_t = emit_scores(b, qb, 0)
                    last = kb == (N // 128) - 1
                    # one-kb software pipelining: the NEXT block's scores+exp
                    # are emitted ahead of this block's PV pair
                    if not last:
                        e_next = emit_scores(b, qb, kb + 1)
                    elif it_idx + 1 < len(iters):
                        e_carry = emit_scores(*iters[it_idx + 1], 0)
                        e_next = None
                    else:
                        e_next = None
                    nc.tensor.matmul(
                        oA[:],
                        vones[:, g, 0:128],
                        e_t[:, 0:512],
                        start=(kb == 0),
                        stop=last,
                    )
                    nc.tensor.matmul(
                        oB[:],
                        vones[:, g, 128:256],
                        e_t[:, 512:1024],
                        start=(kb == 0),
                        stop=last,
                    )
                    if e_next is not None:
                        e_t = e_next
                    for (t, ec) in XB1.get((it_idx, kb), ()):
                        nc.sync.dma_start(
                            x_sb[t][:, ec, :],
                            xT_ext[
                                ec * 128 : (ec + 1) * 128, t * 512 : (t + 1) * 512
                            ],
                        )
                    for fn in FEED.get((it_idx, kb), ()):
                        fn()
                    if (it_idx, kb) == (5, 0):
                        # proj weights: issued mid-attention when HBM is idle
                        nc.sync.dma_start(wp_sb[:], wp_ext[:])
                        nc.sync.dma_start(bias_sb[:], bias_ext[:])
                    if kb == 3 and pending is not None:
                        # one reciprocal covers both heads (dens on
                        # partitions 0 and 32)
                        rec_cur = npool.tile([33, 512], BF, tag="recb", bufs=2)
                        with nc.allow_low_precision(reason="bf16 softmax 1/denom"):
                            nc.vector.reciprocal(rec_cur[:], pending[3][:])
                    if kb == 8 and pending is not None:
                        emit_norm_head(pending, 0, rec_cur)
                    if kb == 1 and it_idx == 7 and pending is not None:
                        # keep a late-written tile as the warm-collective DMA
                        # source so the scheduler cannot hoist the ncfw
                        # warm-up earlier than iteration 7
                        warm_src = pending[2][0]
                    if kb == 12 and pending is not None:
                        emit_norm_head(pending, 1, rec_cur)
                        pending = None
                    if kb == 2 and it_idx == 7 and warm_src is not None:
                        # fires ~12us before the real AllToAll: warms ncfw
                        # AND acts as a barrier that re-syncs core skew so
                        # the AllToAll's entry handshake is short
                        nc.sync.dma_start(warm_in[:], warm_src[64:65, 0:512])
                        nc.gpsimd.collective_compute(
                            "AllGather",
                            ALU.bypass,
                            ins=[warm_in.opt()],
                            outs=[warm_out.opt()],
                            replica_groups=[list(range(NCORES))],
                        )
                # stash raw output + denominators in SBUF so the psum
                # accumulators free; ordered oA-first (den A then raw A) so
                # the next iteration's PV restarts on oA after two DVE ops.
                # On the final iteration the raw copies run on the now-idle
                # Scalar engine so the DVE starts the reciprocal immediately.
                den = npool.tile([33, 512], F32, tag="den", bufs=3)
                raws = []
                final = it_idx == len(iters) - 1
                for j, oX in ((0, oA), (1, oB)):
                    nc.vector.tensor_copy(
                        den[32 * j : 32 * j + 1, :], oX[32 * j : 32 * j + 1, :]
                    )
                    raw = npool.tile([128, 512], BF, tag=f"raw{j}", bufs=3)
                    if final:
                        nc.scalar.copy(raw[64:128, :], oX[64:128, :])
                    else:
                        nc.vector.tensor_copy(raw[64:128, :], oX[64:128, :])
                    raws.append(raw)
                pending = (b, qb, raws, den)

            # tail: one reciprocal for the last iteration, then both norms
            rec_tail = npool.tile([33, 512], BF, tag="recb", bufs=2)
            with nc.allow_low_precision(reason="bf16 softmax 1/denom"):
                nc.vector.reciprocal(rec_tail[:], pending[3][:])
            emit_norm_head(pending, 0, rec_tail)
            emit_norm_head(pending, 1, rec_tail)

            nc.gpsimd.collective_compute(
                "AllToAll",
                ALU.bypass,
                ins=[a2a_in.opt()],
                outs=[a2a_out.opt()],
                replica_groups=[list(range(NCORES))],
            )

            # paced dummy-matmul chain: keeps the PE's activity monitor at
            # full clock across the AllToAll so proj doesn't run at 1.2GHz
            for _ in range(20):
                dps = psum.tile([128, 1024], F32, tag="spair", bufs=3)
                nc.tensor.matmul(
                    dps[:, 0:512], garb[:, 0:128], garb[:, 0:512],
                    start=True, stop=True,
                )
                nc.vector.tensor_copy(gout[:], dps[:, 0:512])

            # ---------------- proj ----------------
            rhs_sb = cpool.tile([128, EC, 512], BF)
            for kc in range(EC):
                nc.sync.dma_start(
                    rhs_sb[:, kc, :], a2a_out[kc * 128 : (kc + 1) * 128, :]
                )
            for ecn in range(EC):
                yp = psum.tile([128, 1024], F32, tag="spair", bufs=3)
                for kc in range(EC):
                    nc.tensor.matmul(
                        yp[:, 0:512],
                        wp_sb[:, kc * 1024 + ecn * 128 : kc * 1024 + (ecn + 1) * 128],
                        rhs_sb[:, kc, :],
                        start=(kc == 0),
                        stop=(kc == EC - 1),
                    )
                y_sb = ypool.tile([128, 512], F32)
                nc.vector.tensor_scalar(
                    out=y_sb[:],
                    in0=yp[:, 0:512],
                    scalar1=bias_sb[:, ecn : ecn + 1],
                    scalar2=None,
                    op0=ALU.add,
                )
                nc.sync.dma_start(out_ext[ecn * 128 : (ecn + 1) * 128, :], y_sb[:])

    _split_multi_waits(nc)
    return nc


def _make_in_maps(x, w_qkv, w_proj, b_proj):
    x = np.asarray(x, dtype=np.float32)
    w_qkv = np.asarray(w_qkv, dtype=np.float32)
    w_proj = np.asarray(w_proj, dtype=np.float32)
    b_proj = np.asarray(b_proj, dtype=np.float32)

    xT = np.ascontiguousarray(x.reshape(TOK, D).T).astype(BF16)
    wq_full = w_qkv[:, 0:D]
    wk_full = w_qkv[:, D : 2 * D]
    wv_full = w_qkv[:, 2 * D : 3 * D]

    def to_sb(wpair):  # [1024, 128] -> [128, 8*128] (e-chunk-major columns)
        return np.ascontiguousarray(
            wpair.reshape(EC, 128, 128).transpose(1, 0, 2).reshape(128, 1024)
        ).astype(BF16)

    wp_sb = np.ascontiguousarray(
        w_proj.reshape(EC, 128, 1024).transpose(1, 0, 2).reshape(128, 8192)
    ).astype(BF16)
    bias_sb = np.ascontiguousarray(b_proj.reshape(EC, 128).T).astype(np.float32)

    in_maps = []
    for c in range(NCORES):
        hA, hB = 2 * c, 2 * c + 1

        def pair(w):
            return np.concatenate(
                [w[:, hA * HD : (hA + 1) * HD], w[:, hB * HD : (hB + 1) * HD]], axis=1
            )

        in_maps.append(
            {
                "xT": xT,
                "wq": to_sb(pair(wq_full)),
                "wk": to_sb(pair(wk_full)),
                "wv": to_sb(pair(wv_full)),
                "wp": wp_sb,
                "bias": bias_sb,
                "coreid": np.array([[c]], dtype=np.uint32),
            }
        )
    return in_maps


_CACHE = {}


def kernel(x, w_qkv, w_proj, b_proj):
    import concourse.bass_utils as bass_utils

    bass_utils.upload_artifacts = lambda tmpdir: tmpdir  # no S3 in container

    if "nc" not in _CACHE:
        _CACHE["nc"] = _build_nc()
    nc = _CACHE["nc"]

    in_maps = _make_in_maps(x, w_qkv, w_proj, b_proj)

    trace = _install_axon_profile_hook()
    try:
        res = bass_utils.run_bass_kernel_spmd(
            nc, in_maps, list(range(NCORES)), trace=trace
        )
    except Exception:
        if not trace:
            raise
        res = bass_utils.run_bass_kernel_spmd(
            nc, in_maps, list(range(NCORES)), trace=False
        )

    kernel.last_exec_time_ns = res.exec_time_ns

    out = np.empty((B, N, D), dtype=np.float32)
    for c in range(NCORES):
        yT = np.asarray(res.results[c]["out"], dtype=np.float32)  # [1024, 512]
        b, s = c // 4, c % 4
        out[b, s * 512 : (s + 1) * 512, :] = yT.T
    return out


kernel.last_exec_time_ns = None


# revision 27
# speedup vs baseline: 1.0695x; 1.0695x over previous
"""Distributed Trainium2 kernel for nn_Attention (B=2, N=2048, D=1024, H=16).

Sharding: tensor-parallel over heads (2 heads per core) for qkv + attention,
then an AllToAll redistributes attention output so each core projects a
512-row slice of the output (cores 0-3: batch 0, cores 4-7: batch 1).

Structure (v6): the exp stream on ScalarE is the attention phase's hard
floor (~1.01us per [128,1024] block, 128 blocks); the Tile scheduler lays
the program out as [init | qkv(b0)+most of qkv(b1) dense | ACT-bound
attention | AllToAll | proj]. Measured lessons folded in:
  - Forcing qkv matmuls between the PV pairs slows the ACT stream ~20%
    (contention), so qkv work is emitted as coarse trailing items and the
    scheduler hoists it into the dense phase.
  - Both heads' softmax denominators land on psum partitions 0/32 (head-B
    ones column at 160), so one DVE reciprocal [33,512] covers both heads.
  - Iteration-boundary ACT bubbles come from the single-buffered PV
    accumulators: copies are ordered denA,rawA,denB,rawB so oA frees after
    two DVE ops; the final iteration's raw copies go to the idle ScalarE.
  - The ncfw warm-up AllGather is pinned late (input DMA sources
    iteration-6 data) so the real AllToAll starts hot.
  - A paced dummy-matmul chain bridges the AllToAll so proj doesn't run
    at the 1.2GHz throttled clock.
"""

import sys
import types

import numpy as np

if "/opt/trn_rl_repo" not in sys.path:
    sys.path.insert(0, "/opt/trn_rl_repo")

import ml_dtypes

B, N, D = 2, 2048, 1024
H, HD = 16, 64
SCALE = HD**-0.5
TOK = B * N  # 4096, token index = b*N + t
EC = 8  # embed-dim chunks of 128
NCORES = 8
# per k-block vones layout [128 tok, 256]:
#   [1 | 0*63 | V_A(64) | 0*32 | 1 | 0*31 | V_B(64)]
# so the PV matmul (M=128) puts head-A's softmax denominator on psum
# partition 0 and head-B's on partition 32.
VSTRIDE = 256
NKB = TOK // 128  # 32 k-blocks across both batches
NTCN = TOK // 512  # 8 qkv token tiles

BF16 = ml_dtypes.bfloat16


def _install_axon_profile_hook():
    """Best-effort: register the NTFF profile hook the RL container's antenv
    stub omits, so run_bass_kernel_spmd(trace=True) can report exec_time_ns."""
    try:
        import antenv

        if "antenv.axon_hooks" not in sys.modules:
            hooks = types.ModuleType("antenv.axon_hooks")
            hooks._hook = None
            hooks.set_axon_ntff_profile_hook = lambda h: setattr(hooks, "_hook", h)
            hooks.get_axon_ntff_profile_hook = lambda: hooks._hook
            sys.modules["antenv.axon_hooks"] = hooks
            antenv.axon_hooks = hooks
            from trn_agent_boot.trn_boot import _ntff_profile_via_ctypes

            hooks.set_axon_ntff_profile_hook(
                _ntff_profile_via_ctypes("/opt/axon/libaxon_pjrt.so")
            )
        return True
    except Exception:
        return False


def _split_multi_waits(nc):
    """neuronxcc's walrus (CoreV3 setupSyncWait) rejects instructions that
    carry more than one semaphore wait, but Tile's wait assignment freely
    attaches several. Hoist the extra waits onto freshly inserted same-engine
    NoOps placed directly before the instruction — the engine stalls at the
    same program point, so semantics are unchanged."""
    import concourse.mybir as mybir

    n_split = 0
    for fn in nc.m.functions:
        for bb in fn.blocks:
            insts = bb.instructions
            if not any(
                i.sync_info is not None and len(i.sync_info.on_wait) > 1
                for i in insts
            ):
                continue
            new_insts = []
            for ins in insts:
                si = ins.sync_info
                if si is not None and len(si.on_wait) > 1:
                    waits = list(si.on_wait)
                    for w in waits[:-1]:
                        nop = mybir.InstNoOp(
                            name=f"wsplit-{n_split}",
                            engine=ins.engine,
                            ins=[],
                            outs=[],
                            sync_info=mybir.SyncInfo(on_wait=[w], on_update=[]),
                        )
                        new_insts.append(nop)
                        n_split += 1
                    ins.sync_info = mybir.SyncInfo(
                        on_wait=[waits[-1]], on_update=list(si.on_update)
                    )
                new_insts.append(ins)
            bb.instructions = new_insts


def _build_nc():
    import concourse.bass as bass
    import concourse.mybir as mybir
    import concourse.tile as tile

    F32 = mybir.dt.float32
    BF = mybir.dt.bfloat16
    AF = mybir.ActivationFunctionType
    ALU = mybir.AluOpType

    nc = bass.Bass()
    xT_ext = nc.declare_dram_parameter("xT", [D, TOK], BF, isOutput=False)
    wq_ext = nc.declare_dram_parameter("wq", [128, 1024], BF, isOutput=False)
    wk_ext = nc.declare_dram_parameter("wk", [128, 1024], BF, isOutput=False)
    wv_ext = nc.declare_dram_parameter("wv", [128, 1024], BF, isOutput=False)
    wp_ext = nc.declare_dram_parameter("wp", [128, 8192], BF, isOutput=False)
    bias_ext = nc.declare_dram_parameter("bias", [128, 8], F32, isOutput=False)
    coreid_ext = nc.declare_dram_parameter(
        "coreid", [1, 1], mybir.dt.uint32, isOutput=False
    )
    out_ext = nc.declare_dram_parameter("out", [D, 512], F32, isOutput=True)

    with tile.TileContext(nc) as tc:
        with (
            tc.tile_pool(name="const", bufs=1) as cpool,
            tc.tile_pool(name="e", bufs=6) as epool,
            tc.tile_pool(name="norm", bufs=2) as npool,
            tc.tile_pool(name="y", bufs=2) as ypool,
            tc.tile_pool(name="psum", bufs=2, space="PSUM") as psum,
            tc.tile_pool(name="dram", bufs=1, space="DRAM") as dram,
        ):
            wq_sb = cpool.tile([128, 1024], BF)
            wk_sb = cpool.tile([128, 1024], BF)
            wv_sb = cpool.tile([128, 1024], BF)
            wp_sb = cpool.tile([128, 8192], BF)
            bias_sb = cpool.tile([128, 8], F32)
            qt_sb = cpool.tile([128, TOK], BF)
            kt_sb = cpool.tile([128, TOK], BF)
            vones = cpool.tile([128, NKB, VSTRIDE], BF)
            x_sb = [
                cpool.tile([128, EC, 512], BF, name=f"x{t}") for t in range(NTCN)
            ]
            garb = cpool.tile([128, 512], BF)
            gout = cpool.tile([128, 512], BF)

            nc.vector.memset(garb[:], 0.0)
            nc.vector.memset(vones[:], 0.0)
            nc.vector.memset(vones[:, :, 0:1], 1.0)
            nc.vector.memset(vones[:, :, 160:161], 1.0)
            bones = cpool.tile([33, 64], BF)
            nc.vector.memset(bones[0:1, :], 1.0)
            nc.vector.memset(bones[32:33, :], 1.0)

            # ---------------- input DMA issue schedule ----------------
            # sync gets the weights + even x(b0) chunks; scalar (idle until
            # the first exp) gets the odd chunks. x(b1) is issued from
            # inside attention iterations 0-1 on sync; wp/bias mid-attention
            # (the prologue window is HBM-bandwidth limited: 8 cores pull
            # replicated x concurrently).
            nc.sync.dma_start(wq_sb[:, 0:512], wq_ext[:, 0:512])
            nc.sync.dma_start(wq_sb[:, 512:1024], wq_ext[:, 512:1024])
            for ec in range(0, EC, 2):
                nc.sync.dma_start(
                    x_sb[0][:, ec, :], xT_ext[ec * 128 : (ec + 1) * 128, 0:512]
                )
            nc.sync.dma_start(wk_sb[:], wk_ext[:])
            nc.sync.dma_start(wv_sb[:], wv_ext[:])
            for tcn in range(1, 4):
                for ec in range(0, EC, 2):
                    nc.sync.dma_start(
                        x_sb[tcn][:, ec, :],
                        xT_ext[ec * 128 : (ec + 1) * 128, tcn * 512 : (tcn + 1) * 512],
                    )
            for tcn in range(4):
                for ec in range(1, EC, 2):
                    nc.scalar.dma_start(
                        x_sb[tcn][:, ec, :],
                        xT_ext[ec * 128 : (ec + 1) * 128, tcn * 512 : (tcn + 1) * 512],
                    )

            # ---------------- qkv emission helpers ----------------
            def emit_QK(t, wsb, dst):
                ps = psum.tile([128, 1024], F32, tag="spair", bufs=3)
                for ec in range(EC):
                    nc.tensor.matmul(
                        ps[:, 0:512],
                        wsb[:, ec * 128 : (ec + 1) * 128],
                        x_sb[t][:, ec, :],
                        start=(ec == 0),
                        stop=(ec == EC - 1),
                    )
                nc.vector.tensor_copy(dst[:, t * 512 : (t + 1) * 512], ps[:, 0:512])

            def emit_Q(t):
                emit_QK(t, wq_sb, qt_sb)

            def emit_K(t):
                emit_QK(t, wk_sb, kt_sb)

            def emit_V(t):
                ps = psum.tile([128, 1024], F32, tag="spair", bufs=3)
                for tsub in range(4):
                    for ec in range(EC):
                        nc.tensor.matmul(
                            ps[:, tsub * 128 : tsub * 128 + 128],
                            x_sb[t][:, ec, tsub * 128 : (tsub + 1) * 128],
                            wv_sb[:, ec * 128 : (ec + 1) * 128],
                            start=(ec == 0),
                            stop=(ec == EC - 1),
                        )
                for tsub in range(4):
                    g = t * 4 + tsub
                    nc.vector.tensor_copy(
                        vones[:, g, 64:128], ps[:, tsub * 128 : tsub * 128 + 64]
                    )
                    nc.vector.tensor_copy(
                        vones[:, g, 192:256], ps[:, tsub * 128 + 64 : tsub * 128 + 128]
                    )

            # ---------------- prologue: minimal batch-0 qkv ----------------
            emit_Q(0)
            emit_K(0)
            emit_V(0)
            emit_K(1)
            emit_V(1)

            # feeder schedule: (iter, kb) -> list of qkv emitters, placed as
            # coarse trailing items; the Tile scheduler hoists them into the
            # dense region where the PE has slack.
            FEED = {
                (0, 0): [lambda: emit_K(2)],
                (0, 3): [lambda: emit_V(2)],
                (0, 6): [lambda: emit_K(3)],
                (0, 9): [lambda: emit_V(3)],
                (0, 13): [lambda: emit_Q(1)],
                (1, 0): [lambda: emit_Q(2)],
                (1, 4): [lambda: emit_K(4)],
                (1, 8): [lambda: emit_V(4)],
                (1, 12): [lambda: emit_Q(3)],
                (2, 0): [lambda: emit_K(5)],
                (2, 4): [lambda: emit_V(5)],
                (2, 8): [lambda: emit_K(6)],
                (3, 0): [lambda: emit_K(7)],
                (3, 4): [lambda: emit_V(6)],
                (3, 8): [lambda: emit_Q(4)],
                (4, 0): [lambda: emit_V(7)],
                (4, 8): [lambda: emit_Q(5)],
                (5, 6): [lambda: emit_Q(6)],
                (6, 6): [lambda: emit_Q(7)],
            }
            # x(b1) chunk DMAs issued on sync from iters 0-1
            XB1 = {}
            for i, t in enumerate(range(4, 8)):
                for ec in range(EC):
                    slot = i * EC + ec  # 0..31 over iters 0-1
                    XB1.setdefault((slot // 16, slot % 16), []).append((t, ec))

            # ---------------- attention ----------------
            a2a_in = dram.tile([1024, 512], BF)
            a2a_out = dram.tile([1024, 512], BF)
            warm_in = dram.tile([1, 512], BF)
            warm_out = dram.tile([8, 512], BF)

            def emit_norm_head(pend, j, rec_in):
                """Normalize one head of a finished (b, qb) iteration's raw
                attention output; overlapped with the next iteration."""
                pb, pqb, raws, _den = pend
                p0 = 32 * j
                bcp = psum.tile([128, 512], F32, tag="spair", bufs=3)
                nc.tensor.matmul(
                    bcp[64:128, :],
                    bones[p0 : p0 + 1, 0:64],
                    rec_in[p0 : p0 + 1, :],
                    start=True,
                    stop=True,
                )
                onorm = npool.tile([128, 512], BF, tag="onorm")
                nc.vector.tensor_mul(
                    onorm[64:128, :], raws[j][64:128, :], bcp[64:128, :]
                )
                row = 128 * (4 * pb + pqb) + 64 * j
                nc.sync.dma_start(a2a_in[row : row + 64, :], onorm[64:128, :])

            def emit_scores(b, qb, kb):
                qoff = b * N + qb * 512
                koff = b * N + kb * 128
                sp = psum.tile([128, 1024], F32, tag="spair", bufs=3)
                nc.tensor.matmul(
                    sp[:, 0:512],
                    kt_sb[0:64, koff : koff + 128],
                    qt_sb[0:64, qoff : qoff + 512],
                    start=True,
                    stop=True,
                )
                nc.tensor.matmul(
                    sp[:, 512:1024],
                    kt_sb[64:128, koff : koff + 128],
                    qt_sb[64:128, qoff : qoff + 512],
                    start=True,
                    stop=True,
                )
                e_t = epool.tile([128, 1024], BF)
                nc.scalar.activation(e_t[:], sp[:], AF.Exp, scale=SCALE)
                return e_t

            iters = [(b, qb) for b in range(B) for qb in range(N // 512)]
            pending = None
            e_carry = None
            rec_cur = None
            warm_src = None
            for it_idx, (b, qb) in enumerate(iters):
                oA = psum.tile([128, 512], F32, tag="oA", bufs=1)
                oB = psum.tile([128, 512], F32, tag="oB", bufs=1)
                for kb in range(N // 128):
                    g = b * (N // 128) + kb
                    if kb == 0:
                        if e_carry is not None:
                            e_t = e_carry
                            e_carry = None
                        else:
                            e_t = emit_scores(b, qb, 0)
                    last = kb == (N // 128) - 1
                    # one-kb software pipelining: the NEXT block's scores+exp
                    # are emitted ahead of this block's PV pair
                    if not last:
                        e_next = emit_scores(b, qb, kb + 1)
                    elif it_idx + 1 < len(iters):
                        e_carry = emit_scores(*iters[it_idx + 1], 0)
                        e_next = None
                    else:
                        e_next = None
                    nc.tensor.matmul(
                        oA[:],
                        vones[:, g, 0:128],
                        e_t[:, 0:512],
                        start=(kb == 0),
                        stop=last,
                    )
                    nc.tensor.matmul(
                        oB[:],
                        vones[:, g, 128:256],
                        e_t[:, 512:1024],
                        start=(kb == 0),
                        stop=last,
                    )
                    if e_next is not None:
                        e_t = e_next
                    for (t, ec) in XB1.get((it_idx, kb), ()):
                        nc.sync.dma_start(
                            x_sb[t][:, ec, :],
                            xT_ext[
                                ec * 128 : (ec + 1) * 128, t * 512 : (t + 1) * 512
                            ],
                        )
                    for fn in FEED.get((it_idx, kb), ()):
                        fn()
                    if (it_idx, kb) == (5, 0):
                        # proj weights: issued mid-attention when HBM is idle
                        nc.sync.dma_start(wp_sb[:], wp_ext[:])
                        nc.sync.dma_start(bias_sb[:], bias_ext[:])
                    if kb == 3 and pending is not None:
                        # one reciprocal covers both heads (dens on
                        # partitions 0 and 32)
                        rec_cur = npool.tile([33, 512], BF, tag="recb", bufs=2)
                        with nc.allow_low_precision(reason="bf16 softmax 1/denom"):
                            nc.vector.reciprocal(rec_cur[:], pending[3][:])
                    if kb == 8 and pending is not None:
                        emit_norm_head(pending, 0, rec_cur)
                    if kb == 10 and it_idx == 6 and pending is not None:
                        # keep a late-written tile as the warm-collective DMA
                        # source so the scheduler cannot hoist the ncfw
                        # warm-up earlier than ~iteration 6
                        warm_src = pending[2][0]
                    if kb == 12 and pending is not None:
                        emit_norm_head(pending, 1, rec_cur)
                        pending = None
                    if kb == 13 and it_idx == 6 and warm_src is not None:
                        nc.sync.dma_start(warm_in[:], warm_src[64:65, 0:512])
                        nc.gpsimd.collective_compute(
                            "AllGather",
                            ALU.bypass,
                            ins=[warm_in.opt()],
                            outs=[warm_out.opt()],
                            replica_groups=[list(range(NCORES))],
                        )
                # stash raw output + denominators in SBUF so the psum
                # accumulators free; ordered oA-first (den A then raw A) so
                # the next iteration's PV restarts on oA after two DVE ops.
                # On the final iteration the raw copies run on the now-idle
                # Scalar engine so the DVE starts the reciprocal immediately.
                den = npool.tile([33, 512], F32, tag="den", bufs=3)
                raws = []
                final = it_idx == len(iters) - 1
                for j, oX in ((0, oA), (1, oB)):
                    nc.vector.tensor_copy(
                        den[32 * j : 32 * j + 1, :], oX[32 * j : 32 * j + 1, :]
                    )
                    raw = npool.tile([128, 512], BF, tag=f"raw{j}", bufs=3)
                    if final:
                        nc.scalar.copy(raw[64:128, :], oX[64:128, :])
                    else:
                        nc.vector.tensor_copy(raw[64:128, :], oX[64:128, :])
                    raws.append(raw)
                pending = (b, qb, raws, den)

            # tail: one reciprocal for the last iteration, then both norms
            rec_tail = npool.tile([33, 512], BF, tag="recb", bufs=2)
            with nc.allow_low_precision(reason="bf16 softmax 1/denom"):
                nc.vector.reciprocal(rec_tail[:], pending[3][:])
            emit_norm_head(pending, 0, rec_tail)
            emit_norm_head(pending, 1, rec_tail)

            nc.gpsimd.collective_compute(
                "AllToAll",
                ALU.bypass,
                ins=[a2a_in.opt()],
                outs=[a2a_out.opt()],
                replica_groups=[list(range(NCORES))],
            )

            # paced dummy-matmul chain: keeps the PE's activity monitor at
            # full clock across the AllToAll so proj doesn't run at 1.2GHz
            for _ in range(20):
                dps = psum.tile([128, 1024], F32, tag="spair", bufs=3)
                nc.tensor.matmul(
                    dps[:, 0:512], garb[:, 0:128], garb[:, 0:512],
                    start=True, stop=True,
                )
                nc.vector.tensor_copy(gout[:], dps[:, 0:512])

            # ---------------- proj ----------------
            rhs_sb = cpool.tile([128, EC, 512], BF)
            for kc in range(EC):
                nc.sync.dma_start(
                    rhs_sb[:, kc, :], a2a_out[kc * 128 : (kc + 1) * 128, :]
                )
            for ecn in range(EC):
                yp = psum.tile([128, 1024], F32, tag="spair", bufs=3)
                for kc in range(EC):
                    nc.tensor.matmul(
                        yp[:, 0:512],
                        wp_sb[:, kc * 1024 + ecn * 128 : kc * 1024 + (ecn + 1) * 128],
                        rhs_sb[:, kc, :],
                        start=(kc == 0),
                        stop=(kc == EC - 1),
                    )
                y_sb = ypool.tile([128, 512], F32)
                nc.vector.tensor_scalar(
                    out=y_sb[:],
                    in0=yp[:, 0:512],
                    scalar1=bias_sb[:, ecn : ecn + 1],
                    scalar2=None,
                    op0=ALU.add,
                )
                nc.sync.dma_start(out_ext[ecn * 128 : (ecn + 1) * 128, :], y_sb[:])

    _split_multi_waits(nc)
    return nc


def _make_in_maps(x, w_qkv, w_proj, b_proj):
    x = np.asarray(x, dtype=np.float32)
    w_qkv = np.asarray(w_qkv, dtype=np.float32)
    w_proj = np.asarray(w_proj, dtype=np.float32)
    b_proj = np.asarray(b_proj, dtype=np.float32)

    xT = np.ascontiguousarray(x.reshape(TOK, D).T).astype(BF16)
    wq_full = w_qkv[:, 0:D]
    wk_full = w_qkv[:, D : 2 * D]
    wv_full = w_qkv[:, 2 * D : 3 * D]

    def to_sb(wpair):  # [1024, 128] -> [128, 8*128] (e-chunk-major columns)
        return np.ascontiguousarray(
            wpair.reshape(EC, 128, 128).transpose(1, 0, 2).reshape(128, 1024)
        ).astype(BF16)

    wp_sb = np.ascontiguousarray(
        w_proj.reshape(EC, 128, 1024).transpose(1, 0, 2).reshape(128, 8192)
    ).astype(BF16)
    bias_sb = np.ascontiguousarray(b_proj.reshape(EC, 128).T).astype(np.float32)

    in_maps = []
    for c in range(NCORES):
        hA, hB = 2 * c, 2 * c + 1

        def pair(w):
            return np.concatenate(
                [w[:, hA * HD : (hA + 1) * HD], w[:, hB * HD : (hB + 1) * HD]], axis=1
            )

        in_maps.append(
            {
                "xT": xT,
                "wq": to_sb(pair(wq_full)),
                "wk": to_sb(pair(wk_full)),
                "wv": to_sb(pair(wv_full)),
                "wp": wp_sb,
                "bias": bias_sb,
                "coreid": np.array([[c]], dtype=np.uint32),
            }
        )
    return in_maps


_CACHE = {}


def kernel(x, w_qkv, w_proj, b_proj):
    import concourse.bass_utils as bass_utils

    bass_utils.upload_artifacts = lambda tmpdir: tmpdir  # no S3 in container

    if "nc" not in _CACHE:
        _CACHE["nc"] = _build_nc()
    nc = _CACHE["nc"]

    in_maps = _make_in_maps(x, w_qkv, w_proj, b_proj)

    trace = _install_axon_profile_hook()
    try:
        res = bass_utils.run_bass_kernel_spmd(
            nc, in_maps, list(range(NCORES)), trace=trace
        )
    except Exception:
        if not trace:
            raise
        res = bass_utils.run_bass_kernel_spmd(
            nc, in_maps, list(range(NCORES)), trace=False
        )

    kernel.last_exec_time_ns = res.exec_time_ns

    out = np.empty((B, N, D), dtype=np.float32)
    for c in range(NCORES):
        yT = np.asarray(res.results[c]["out"], dtype=np.float32)  # [1024, 512]
        b, s = c // 4, c % 4
        out[b, s * 512 : (s + 1) * 512, :] = yT.T
    return out


kernel.last_exec_time_ns = None
